# revision 2
# baseline (speedup 1.0000x reference)
"""DiDi attention Trainium2 kernel, v2.

Reference (per batch b):
    ua[s] = A[b,s,:] @ u_w ;  vl[t] = L[b,t,:] @ v_w + v_b
    score[t,s] = tanh(vl[t] + ua[s]) * mask_a[s]
    norm[t] = sum_s score[t,s]  (1 on padded rows)
    out[b,t,:] = (score[t,:] @ A[b]) / norm[t] * mask_l[t]

The device computes the O(Sl*Sa*Da) heart: the tanh score tiles and
the score@[A|mask] contraction, in float32r (measured: RNE to 11
mantissa bits on input, then an exact matmul at 1 cycle/row -- 4x the
fp32 rate).  The host computes the two skinny projections ua/vl (0.1%
of FLOPs), plans the work, and divides num/norm during the gather (the
previous version already divided on host).

Numerics: the normalizer is a signed tanh sum whose smallest values
(~1e-2) dominate the L2 error metric, and an 11-bit score sum is only
safe where |norm| >~ 0.45.  The host predicts the at-risk rows with a
spline of f_b(v) = sum_s tanh(v + ua[s]) (norm[t] = f_b(vl[t]),
monotone and smooth, so a 257-node spline flags reliably with margin)
and computes exact fp32 norms for just those rows (~85 of 15399 here,
~9k tanh evaluations) itself.  Full-pipeline simulation of exactly
this arithmetic: rel err 1.63e-3 against the fp32 reference.

Work layout: batch b is tl_b t-columns (x128 rows) of depth ta_b
a-tiles.  A fragment may take any subset of one batch's columns (pv
and outputs gather/scatter host-side) and any a-tile subrange
(partials sum host-side).  The shared static program is a list of
(depth D, width W<=8) slots; W<=8 because each t-tile's [128,258]
output (256 features + mask column for the normalizer) occupies one
PSUM bank.  Depth bands are merged agglomeratively, trading slot
padding (~252ns/pair) against fragment count (extra output DMA).
"""

import os
import sys
import types
from collections import deque

sys.path.insert(0, '/opt/trn_rl_repo')
os.environ.setdefault('JAX_PLATFORMS', 'cpu')

try:
    from antenv.axon_hooks import get_axon_ntff_profile_hook  # noqa: F401
except ImportError:
    _m = types.ModuleType('antenv.axon_hooks')
    _hook_slot = [None]
    _m.set_axon_ntff_profile_hook = lambda h: _hook_slot.__setitem__(0, h)
    _m.get_axon_ntff_profile_hook = lambda: _hook_slot[0]
    sys.modules['antenv.axon_hooks'] = _m
    import antenv
    antenv.axon_hooks = _m
    try:
        from trn_agent_boot.trn_boot import _ntff_profile_via_ctypes
        _m.set_axon_ntff_profile_hook(
            _ntff_profile_via_ctypes('/opt/axon/libaxon_pjrt.so'))
    except Exception:
        pass

import numpy as np

import bass_rust
import concourse.bass as bass
import concourse.tile as tile
from concourse import mybir
from concourse.bass_utils import run_bass_kernel_spmd

NCORES = 8
PT = 128
DA = 256
NAUG = 258        # 256 features + mask col + pad to even
WMAX = 8          # t-tiles per slot: one PSUM bank per 258-col tile
FLAG_THR = 0.45   # |norm| below this -> host-exact norm
F32 = mybir.dt.float32
F32R = mybir.dt.float32r

last_perf = {}


def _fixup_waits(nc, maxw=1):
    """Split >1-semaphore waits onto NOP carriers (walrus build limit)."""
    n = 0
    for f in nc.m.functions:
        for blk in f.blocks:
            insts = list(blk.instructions)
            out = []
            changed = False
            for inst in insts:
                si = inst.sync_info
                if si is not None and len(si.on_wait) > maxw:
                    waits = list(si.on_wait)
                    head, keep = waits[:-maxw], waits[-maxw:]
                    for j in range(0, len(head), maxw):
                        nop = mybir.InstNoOp(name=f"WSPLIT-{n}", ins=[], outs=[])
                        n += 1
                        nop.engine = inst.engine
                        nop.sync_info = bass_rust.SyncInfo(
                            on_wait=head[j:j + maxw], on_update=[])
                        out.append(nop)
                    si.on_wait = keep
                    inst.sync_info = si
                    changed = True
                out.append(inst)
            if changed:
                blk.instructions = out
    return n


# ----------------------------------------------------------------- planner

def _bands_from_bounds(bounds):
    """bounds ascending e.g. [4,8,12,16] -> bands descending [(16,12),...]"""
    bs = [0] + list(bounds)
    return [(bs[i + 1], bs[i]) for i in range(len(bs) - 1)][::-1]


def _plan_bands(ta, tl, bands):
    B = len(ta)
    pieces = []
    for top, bot in bands:
        for b in range(B):
            if ta[b] <= bot or tl[b] == 0:
                continue
            d_eff = min(ta[b], top) - bot
            nch = -(-tl[b] // WMAX)
            base, rem = divmod(tl[b], nch)
            t0 = 0
            for i in range(nch):
                w = base + (1 if i < rem else 0)
                pieces.append((w, d_eff, b, bot, list(range(t0, t0 + w))))
                t0 += w
    pieces.sort(key=lambda p: (-p[1], -p[0]))
    slots = []
    frags = [[] for _ in range(NCORES)]
    load = [0.0] * NCORES
    for r in range(0, len(pieces), NCORES):
        grp = pieces[r:r + NCORES]
        D = max(p[1] for p in grp)
        W = max(p[0] for p in grp)
        order = sorted(range(NCORES), key=lambda c: load[c])
        pc = {}
        for rank, p in enumerate(sorted(grp, key=lambda p: -p[0] * p[1])):
            c = order[rank]
            pc[c] = (p[2], p[3], p[1], p[4])
            load[c] += p[0] * p[1]
        slots.append((D, W))
        for c in range(NCORES):
            frags[c].append(pc.get(c))
    return slots, frags


def _plan(length_a, length_l):
    """Static slot list + per-core fragments, best over candidate band sets.

    Returns (slots, frags): slots = [(D, W)]; frags[c][j] is None or
    (b, s0, d, [t...]): batch, a-tile start, real depth, t-tile list.
    """
    ta = [-(-int(x) // PT) for x in length_a]
    tl = [-(-int(x) // PT) for x in length_l]
    dmax = max(ta)
    cands = []
    for k in (2, 3, 4, 5, 6, 8, dmax):
        cands.append(list(range(k, dmax, k)) + [dmax])
    depths = sorted({d for d in ta if d > 0})
    cands.append(depths)
    for k in (3, 4, 5):   # quantile-ish splits of distinct depths
        if len(depths) > k:
            idx = [int(round(i * (len(depths) - 1) / (k - 1)))
                   for i in range(k)]
            cands.append(sorted({depths[i] for i in idx} | {dmax}))

    best = None
    for bounds in cands:
        bands = _bands_from_bounds(bounds)
        slots, frags = _plan_bands(ta, tl, bands)
        padded = sum(D * W for D, W in slots)
        sumd = sum(D for D, _ in slots)
        sumw = sum(W for _, W in slots)
        cost = 252.0 * padded + 466.0 * sumd + 250.0 * sumw             + 150.0 * (sumd + sumw)
        if best is None or cost < best[0]:
            best = (cost, slots, frags)
    _, slots, frags = best
    # deep-first, with one shallow slot leading to warm the pipeline
    order = sorted(range(len(slots)), key=lambda j: -slots[j][0] * slots[j][1])
    if len(order) > 2:
        order = [order[-2]] + order[:-2] + [order[-1]]
    slots = [slots[j] for j in order]
    frags = [[row[j] for j in order] for row in frags]
    return slots, frags


def _host_norms(ua, vl, length_a, length_l):
    """Flag at-risk rows via spline of f_b, compute their norms in fp32."""
    B = len(length_a)
    flagged, norms = [], []
    for b in range(B):
        la, ll = int(length_a[b]), int(length_l[b])
        uab = ua[b][:la]
        v = vl[b][:ll]
        nodes = np.linspace(v.min() - 0.1, v.max() + 0.1, 257,
                            dtype=np.float32)
        fvals = np.tanh(nodes[:, None] + uab[None, :]).sum(-1)
        fap = np.interp(v, nodes, fvals)
        rows = np.nonzero(np.abs(fap) < FLAG_THR)[0]
        flagged.append(rows)
        if len(rows):
            norms.append(np.tanh(v[rows][:, None] + uab[None, :])
                         .astype(np.float32).sum(-1, dtype=np.float32))
        else:
            norms.append(np.zeros(0, np.float32))
    return flagged, norms


# ----------------------------------------------------------------- device

def _build(slots):
    nc = bass.Bass()
    sumd = sum(d for d, _ in slots)
    sumw = sum(w for _, w in slots)

    a_d = nc.dram_tensor("a_aug", [sumd, PT, NAUG], F32R, kind="ExternalInput")
    ua_d = nc.dram_tensor("ua", [PT, sumd], F32, kind="ExternalInput")
    pv_d = nc.dram_tensor("pv", [PT, sumw * PT], F32, kind="ExternalInput")
    out_d = nc.dram_tensor("out", [sumw, PT, NAUG], F32, kind="ExternalOutput")

    aq = [nc.gpsimd, nc.scalar]
    oq = [nc.sync, nc.gpsimd, nc.scalar]

    with tile.TileContext(nc) as tc:
        with (
            tc.tile_pool(name="aa", bufs=2) as aa_pool,
            tc.tile_pool(name="uap", bufs=1) as ua_pool,
            tc.tile_pool(name="pvp", bufs=2) as pv_pool,
            tc.tile_pool(name="scop", bufs=3) as sco_pool,
            tc.tile_pool(name="otp", bufs=2) as ot_pool,
            tc.tile_pool(name="psp", bufs=1, space="PSUM") as ps_pool,
        ):
            uat = ua_pool.tile([PT, sumd], F32)
            nc.sync.dma_start(uat[:], ua_d[:, :])
            qi = 0
            oi = 0

            aoff = 0
            woff = 0
            for j, (D, W) in enumerate(slots):
                aaj = aa_pool.tile([PT, 16, NAUG], F32R, tag="aaj")
                for ss in range(D):
                    aq[qi % len(aq)].dma_start(
                        aaj[:, ss, :], a_d[aoff + ss, :, :])
                    qi += 1
                pvj = pv_pool.tile([PT, WMAX * PT], F32, tag="pvj")
                nc.sync.dma_start(pvj[:, 0:W * PT],
                                  pv_d[:, woff * PT:(woff + W) * PT])
                po = ps_pool.tile([PT, WMAX, 512], F32, tag="po")
                for ss in range(D):
                    sco = sco_pool.tile([PT, WMAX * PT], F32R, tag="sco")
                    nc.scalar.activation(
                        sco[:, 0:W * PT], pvj[:, 0:W * PT],
                        mybir.ActivationFunctionType.Tanh,
                        bias=uat[:, aoff + ss:aoff + ss + 1], scale=1.0)
                    for w in range(W):
                        nc.tensor.matmul(
                            po[:, w, 0:NAUG],
                            sco[:, w * PT:(w + 1) * PT],
                            aaj[:, ss, :],
                            start=(ss == 0), stop=(ss == D - 1))
                ot = ot_pool.tile([PT, WMAX, NAUG], F32, tag="ot")
                nc.vector.tensor_copy(ot[:, 0:W, :], po[:, 0:W, 0:NAUG])
                for w in range(W):
                    oq[oi % len(oq)].dma_start(out_d[woff + w, :, :],
                                               ot[:, w, :])
                    oi += 1
                aoff += D
                woff += W

    _fixup_waits(nc)
    return nc


# ------------------------------------------------------------------- host

def kernel(A, L, length_a, length_l, u_w, v_w, v_b):
    A = np.ascontiguousarray(np.asarray(A, dtype=np.float32))
    L = np.ascontiguousarray(np.asarray(L, dtype=np.float32))
    length_a = np.asarray(length_a, dtype=np.int32)
    length_l = np.asarray(length_l, dtype=np.int32)
    u_w = np.asarray(u_w, dtype=np.float32)
    v_w = np.asarray(v_w, dtype=np.float32)
    v_b = np.asarray(v_b, dtype=np.float32)
    B, SL, _ = L.shape

    ua = np.einsum('bsd,d->bs', A, u_w[0]).astype(np.float32)
    vl = (np.einsum('btd,d->bt', L, v_w[0]) + v_b[0]).astype(np.float32)

    slots, frags = _plan(length_a, length_l)
    flagged, flag_norms = _host_norms(ua, vl, length_a, length_l)

    nc = _build(slots)

    sumd = sum(d for d, _ in slots)
    sumw = sum(w for _, w in slots)

    in_maps = []
    for c in range(NCORES):
        a_aug = np.zeros((sumd, PT, NAUG), np.float32)
        ua_t = np.zeros((PT, sumd), np.float32)
        pv_t = np.zeros((PT, sumw * PT), np.float32)
        aoff = woff = 0
        for j, (D, W) in enumerate(slots):
            fr = frags[c][j]
            if fr is not None:
                b, s0, d, ts = fr
                la = int(length_a[b])
                lo = s0 * PT
                hi = min((s0 + d) * PT, la)
                if hi > lo:
                    blk = np.zeros((d * PT, NAUG), np.float32)
                    blk[0:hi - lo, 0:DA] = A[b, lo:hi]
                    blk[0:hi - lo, DA] = 1.0
                    a_aug[aoff:aoff + d] = blk.reshape(d, PT, NAUG)
                    uacol = np.zeros(d * PT, np.float32)
                    uacol[0:hi - lo] = ua[b, lo:hi]
                    ua_t[:, aoff:aoff + d] = uacol.reshape(d, PT).T
                for wi, t in enumerate(ts):
                    te = min((t + 1) * PT, SL)
                    seg = vl[b, t * PT:te]
                    pv_t[:, (woff + wi) * PT:(woff + wi) * PT + len(seg)] = \
                        seg[None, :]
            aoff += D
            woff += W
        in_maps.append({"a_aug": a_aug, "ua": ua_t, "pv": pv_t})

    trace = os.environ.get("BASS_DIDI_TRACE") == "1"
    res = run_bass_kernel_spmd(
        nc, in_maps, core_ids=list(range(NCORES)), trace=trace)
    if trace:
        last_perf.clear()
        last_perf.update(
            exec_time_ns=res.exec_time_ns,
            mean_exec_time_ns=res.mean_exec_time_ns,
            trace=res.instructions_and_trace[1]
            if res.instructions_and_trace else None)

    # gather: sum depth-partials; host-exact norms for flagged rows
    num = np.zeros((B, SL, DA), np.float32)
    nrm = np.zeros((B, SL), np.float32)
    for c in range(NCORES):
        o = res.results[c]["out"]
        woff = 0
        for j, (D, W) in enumerate(slots):
            fr = frags[c][j]
            if fr is not None:
                b, s0, d, ts = fr
                ll = int(length_l[b])
                for wi, t in enumerate(ts):
                    nv = min(PT, ll - t * PT)
                    if nv <= 0:
                        continue
                    tile_o = o[woff + wi]
                    num[b, t * PT:t * PT + nv] += tile_o[:nv, 0:DA]
                    nrm[b, t * PT:t * PT + nv] += tile_o[:nv, DA]
            woff += W
    for b in range(B):
        if len(flagged[b]):
            nrm[b, flagged[b]] = flag_norms[b]

    out = np.zeros((B, SL, DA), np.float32)
    for b in range(B):
        ll = int(length_l[b])
        dnm = np.where(np.abs(nrm[b, :ll]) > 0, nrm[b, :ll], 1.0)
        out[b, :ll] = num[b, :ll] / dnm[:, None]
    return out
